# revision 4
# baseline (speedup 1.0000x reference)
"""MoE block (AdaptFormer adapters, top-2 of 8 experts) on 8 TRN2 NeuronCores.

Strategy: data-parallel over the 8192 tokens (1024 tokens/core), router +
expert adapter weights replicated on every core. Per core:
  - PE-transpose x tiles in fp32 (bit-exact) to get xT with D on partitions.
  - logits = x @ w_gate in exact fp32 with w_gate stationary (8-wide
    LDWEIGHTS, negligible): exact logits so top-2 selection matches the
    fp32 reference ordering; computed transposed [8, tok] then PE-transposed
    back for the gating math.
  - h = x @ Wd_all in float32r (full-rate matmul, ~2e-4 rel err), relu,
    multiply by 0.5 * top2-softmax gates (DVE/ACT, from exact logits).
  - out = (g .* h) @ Wu_flat over the concatenated 512-wide expert axis.
All dense: every expert computed for every token, sparse gates zero the
rest (mathematically identical to dispatch/combine).
"""
import numpy as np
from contextlib import ExitStack

import concourse.bass as bass
import concourse.tile as tile
from concourse import bacc, mybir
from concourse.bass_utils import run_bass_kernel_spmd

N_CORES = 8
B_DIM, S_DIM, D = 2, 4096, 1024
T = B_DIM * S_DIM          # 8192 tokens
TC = T // N_CORES          # 1024 tokens per core
E, BK = 8, 64              # experts, bottleneck
EB = E * BK                # 512 concatenated expert axis
P = 128
NTT = TC // P              # token tiles per core
KC = D // P                # D chunks
BC = EB // P               # bottleneck chunks
SCALE = 0.5
N_WARM = 24                # PE warm-up matmuls during initial DMA wait

F32 = mybir.dt.float32
F32R = mybir.dt.float32r
AL = mybir.AluOpType
ACTF = mybir.ActivationFunctionType
AX = mybir.AxisListType

_BUILD_CACHE = {}


def _build(include_bd: bool, include_bu: bool, reps: int = 1):
    key = (include_bd, include_bu, reps)
    if key in _BUILD_CACHE:
        return _BUILD_CACHE[key]

    nc = bacc.Bacc("TRN2", target_bir_lowering=False, debug=False,
                   num_devices=N_CORES)
    x_d = nc.dram_tensor("x", [TC, D], F32, kind="ExternalInput").ap()
    wd_d = nc.dram_tensor("wd", [D, EB], F32, kind="ExternalInput").ap()
    wu_d = nc.dram_tensor("wu", [EB, D], F32, kind="ExternalInput").ap()
    wg_d = nc.dram_tensor("wg", [D, E], F32, kind="ExternalInput").ap()
    id_d = nc.dram_tensor("ident", [P, P], F32, kind="ExternalInput").ap()
    if include_bd:
        bd_d = nc.dram_tensor("bd", [1, EB], F32, kind="ExternalInput").ap()
    if include_bu:
        bu_d = nc.dram_tensor("bu", [E, D], F32, kind="ExternalInput").ap()
    out_d = nc.dram_tensor("out", [TC, D], F32, kind="ExternalOutput").ap()

    with tile.TileContext(nc) as tc, ExitStack() as ctx:
        wpool = ctx.enter_context(tc.tile_pool(name="weights", bufs=1))
        xpool = ctx.enter_context(tc.tile_pool(name="x", bufs=4))
        xtpool = ctx.enter_context(tc.tile_pool(name="xt", bufs=2))
        hgpool = ctx.enter_context(tc.tile_pool(name="hg", bufs=2))
        gpool = ctx.enter_context(tc.tile_pool(name="gates", bufs=2))
        opool = ctx.enter_context(tc.tile_pool(name="osb", bufs=3))

        xt_ps_pool = ctx.enter_context(
            tc.tile_pool(name="xtps", bufs=2, space="PSUM"))
        h_ps_pool = ctx.enter_context(
            tc.tile_pool(name="hps", bufs=2, space="PSUM"))
        lt_ps_pool = ctx.enter_context(
            tc.tile_pool(name="ltps", bufs=1, space="PSUM"))
        lb_ps_pool = ctx.enter_context(
            tc.tile_pool(name="lbps", bufs=1, space="PSUM"))
        hgt_ps_pool = ctx.enter_context(
            tc.tile_pool(name="hgtps", bufs=1, space="PSUM"))
        o_ps_pool = ctx.enter_context(
            tc.tile_pool(name="ops", bufs=1, space="PSUM"))

        # identity first (small, unblocks transposes + warm-up)
        ident = wpool.tile([P, P], F32, tag="ident")
        nc.sync.dma_start(ident[:], id_d)
        ident_r = wpool.tile([P, P], F32R, tag="identr")
        nc.sync.dma_start(ident_r[:], id_d.bitcast(F32R))

        # PE warm-up: real (non-transpose) f32r matmuls to trip the HAM
        # un-throttle while the first DMAs stream in.
        warm_ps = hgt_ps_pool.tile([P, P], F32R, tag="hgtps")
        for i in range(N_WARM):
            nc.tensor.matmul(warm_ps[:].bitcast(F32), ident_r[:], ident_r[:],
                             start=True, stop=True)

        wd_sb = [wpool.tile([P, EB], F32R, tag=f"wd{c}", name=f"wd{c}")
                 for c in range(KC)]
        wu_sb = [wpool.tile([P, D], F32R, tag=f"wu{k}", name=f"wu{k}")
                 for k in range(BC)]
        wg_sb = wpool.tile([P, KC, E], F32, tag="wg")
        if include_bd:
            ones_r = wpool.tile([1, P], F32R, tag="ones")
            nc.vector.memset(ones_r[:], 1.0)
            bd_sb = wpool.tile([1, EB], F32R, tag="bd")
        if include_bu:
            bu_sb = wpool.tile([E, D], F32R, tag="bu")

        for rep in range(reps):
            for t in range(NTT):
                first = (rep == 0 and t == 0)
                rows = bass.ts(t, P)
                x_t = xpool.tile([P, D], F32, tag="x")
                nc.sync.dma_start(x_t[:], x_d[rows, :])

                if first:
                    # weights stream in behind x(0)/ident; chunked so each
                    # consumer only waits for its own slice.
                    for c in range(KC):
                        nc.sync.dma_start(
                            wd_sb[c][:],
                            wd_d.bitcast(F32R)[bass.ts(c, P), :])
                    nc.sync.dma_start(
                        wg_sb[:], wg_d.rearrange("(c p) n -> p c n", p=P))
                    for k in range(BC):
                        nc.sync.dma_start(
                            wu_sb[k][:],
                            wu_d.bitcast(F32R)[bass.ts(k, P), :])
                    if include_bd:
                        nc.sync.dma_start(bd_sb[:], bd_d.bitcast(F32R))
                    if include_bu:
                        nc.sync.dma_start(bu_sb[:], bu_d.bitcast(F32R))

                # transpose x tile: 8 chunks [128tok,128d] -> [128d,128tok]
                xt32 = xtpool.tile([P, KC, P], F32, tag="xt32")
                xtr = xtpool.tile([P, KC, P], F32R, tag="xtr")
                for h2 in range(2):
                    xt_ps = xt_ps_pool.tile([P, 4 * P], F32, tag="xtps")
                    for c4 in range(4):
                        c = 4 * h2 + c4
                        nc.tensor.transpose(
                            xt_ps[:, bass.ts(c4, P)],
                            x_t[:, bass.ts(c, P)], ident[:])
                    xt_ps3 = xt_ps[:].rearrange("p (c m) -> p c m", c=4)
                    nc.scalar.copy(xt32[:, 4 * h2:4 * h2 + 4, :], xt_ps3)
                    nc.vector.tensor_copy(xtr[:, 4 * h2:4 * h2 + 4, :], xt_ps3)

                # exact fp32 logits, transposed: lT[8, tok]
                lt_ps = lt_ps_pool.tile([E, P], F32, tag="ltps")
                for c in range(KC):
                    nc.tensor.matmul(lt_ps[:], wg_sb[:, c, :], xt32[:, c, :],
                                     start=(c == 0), stop=(c == KC - 1))
                lt_sb = gpool.tile([E, P], F32, tag="ltsb")
                nc.scalar.copy(lt_sb[:], lt_ps[:])
                # transpose back to [tok, 8] for the gating math
                lb_ps = lb_ps_pool.tile([P, E], F32, tag="lbps")
                nc.tensor.transpose(lb_ps[:], lt_sb[:], ident[0:E, 0:E])
                l_sb = gpool.tile([P, E], F32, tag="lsb")
                nc.scalar.copy(l_sb[:], lb_ps[:])

                # step A: h = x @ Wd_all in f32r
                h_ps = h_ps_pool.tile([P, EB], F32, tag="hps")
                n_a = KC + (1 if include_bd else 0)
                for c in range(KC):
                    nc.tensor.matmul(h_ps[:], xtr[:, c, :], wd_sb[c][:],
                                     start=(c == 0), stop=(c == n_a - 1))
                if include_bd:
                    nc.tensor.matmul(h_ps[:], ones_r[:], bd_sb[:],
                                     start=False, stop=True)

                # gating: g2 = 0.5 * scatter(softmax(top2(logits)))
                m1 = gpool.tile([P, 1], F32, tag="m1")
                nc.vector.tensor_reduce(m1[:], l_sb[:], AX.X, AL.max)
                m1n = gpool.tile([P, 1], F32, tag="m1n")
                nc.vector.tensor_scalar_mul(m1n[:], m1[:], -1.0)
                mask1 = gpool.tile([P, E], F32, tag="mask1")
                nc.vector.tensor_scalar(mask1[:], l_sb[:], m1[:], None,
                                        op0=AL.is_ge)
                lm = gpool.tile([P, E], F32, tag="lm")
                nc.vector.scalar_tensor_tensor(lm[:], mask1[:], -1e30, l_sb[:],
                                               op0=AL.mult, op1=AL.add)
                m2 = gpool.tile([P, 1], F32, tag="m2")
                nc.vector.tensor_reduce(m2[:], lm[:], AX.X, AL.max)
                e2 = gpool.tile([P, 1], F32, tag="e2")
                nc.scalar.activation(e2[:], m2[:], ACTF.Exp, bias=m1n[:])
                d2 = gpool.tile([P, 1], F32, tag="d2")
                nc.scalar.activation(d2[:], e2[:], ACTF.Copy,
                                     bias=1.0 / SCALE, scale=1.0 / SCALE)
                rh = gpool.tile([P, 1], F32, tag="rh")
                nc.vector.reciprocal(rh[:], d2[:])
                expl = gpool.tile([P, E], F32, tag="expl")
                nc.scalar.activation(expl[:], l_sb[:], ACTF.Exp, bias=m1n[:])
                mask2 = gpool.tile([P, E], F32, tag="mask2")
                nc.vector.tensor_scalar(mask2[:], l_sb[:], m2[:], None,
                                        op0=AL.is_ge)
                g2 = gpool.tile([P, E], F32, tag="g2")
                nc.vector.scalar_tensor_tensor(g2[:], expl[:], rh[:], mask2[:],
                                               op0=AL.mult, op1=AL.mult)

                # relu + gate multiply -> HG (f32r)
                r_t = hgpool.tile([P, EB], F32, tag="relu")
                nc.scalar.activation(r_t[:], h_ps[:], ACTF.Relu)
                hg = hgpool.tile([P, EB], F32R, tag="hg")
                nc.vector.tensor_tensor(
                    hg[:].rearrange("p (e j) -> p e j", e=E),
                    r_t[:].rearrange("p (e j) -> p e j", e=E),
                    g2[:].unsqueeze(2).broadcast_to([P, E, BK]),
                    op=AL.mult)

                # transpose HG (f32r) -> HGT chunks
                hgt_ps = hgt_ps_pool.tile([P, EB], F32R, tag="hgtps")
                for k in range(BC):
                    nc.tensor.transpose(hgt_ps[:, bass.ts(k, P)],
                                        hg[:, bass.ts(k, P)], ident_r[:])
                hgt = hgpool.tile([P, BC, P], F32R, tag="hgt")
                nc.scalar.copy(
                    hgt[:], hgt_ps[:].rearrange("p (c m) -> p c m", c=BC))

                if include_bu:
                    g2r = gpool.tile([P, E], F32R, tag="g2r")
                    nc.vector.tensor_copy(g2r[:], g2[:])
                    g2t_ps = lt_ps_pool.tile([E, P], F32R, tag="g2tps")
                    nc.tensor.transpose(g2t_ps[:], g2r[:], ident_r[:])
                    g2t = gpool.tile([E, P], F32R, tag="g2t")
                    nc.scalar.copy(g2t[:], g2t_ps[:])

                # step B: out = HG @ Wu_flat (+ g2 @ bu)
                for h in range(2):
                    o_ps = o_ps_pool.tile([P, 512], F32, tag="ops")
                    n_b = BC + (1 if include_bu else 0)
                    for k in range(BC):
                        nc.tensor.matmul(
                            o_ps[:], hgt[:, k, :],
                            wu_sb[k][:, bass.ts(h, 512)],
                            start=(k == 0), stop=(k == n_b - 1))
                    if include_bu:
                        nc.tensor.matmul(o_ps[:], g2t[:],
                                         bu_sb[:, bass.ts(h, 512)],
                                         start=False, stop=True)
                    o_sb = opool.tile([P, 512], F32, tag="osb")
                    if h == 0:
                        nc.vector.tensor_copy(o_sb[:], o_ps[:])
                    else:
                        nc.scalar.copy(o_sb[:], o_ps[:])
                    nc.sync.dma_start(out_d[rows, bass.ts(h, 512)], o_sb[:])

    nc.compile()
    _BUILD_CACHE[key] = nc
    return nc


def kernel(x, w_gate, w_noise, Wd, bd, Wu, bu, reps: int = 1):
    x = np.ascontiguousarray(np.asarray(x, dtype=np.float32))
    assert x.shape == (B_DIM, S_DIM, D), x.shape
    wg = np.ascontiguousarray(np.asarray(w_gate, dtype=np.float32))
    Wd = np.asarray(Wd, dtype=np.float32)
    Wu = np.asarray(Wu, dtype=np.float32)
    bd = np.asarray(bd, dtype=np.float32)
    bu = np.asarray(bu, dtype=np.float32)

    include_bd = bool(np.any(bd))
    include_bu = bool(np.any(bu))
    nc = _build(include_bd, include_bu, reps)

    wd_all = np.ascontiguousarray(
        Wd.transpose(1, 0, 2).reshape(D, EB))          # [D, E*BK]
    wu_flat = np.ascontiguousarray(Wu.reshape(EB, D))  # [E*BK, D]
    ident = np.eye(P, dtype=np.float32)

    xf = x.reshape(T, D)
    shared = dict(wd=wd_all, wu=wu_flat, wg=wg, ident=ident)
    if include_bd:
        shared["bd"] = np.ascontiguousarray(bd.reshape(1, EB))
    if include_bu:
        shared["bu"] = np.ascontiguousarray(bu)
    in_maps = [
        dict(x=np.ascontiguousarray(xf[c * TC:(c + 1) * TC]), **shared)
        for c in range(N_CORES)
    ]
    res = run_bass_kernel_spmd(nc, in_maps, core_ids=list(range(N_CORES)))
    out = np.concatenate([res.results[c]["out"] for c in range(N_CORES)], axis=0)
    return out.reshape(B_DIM, S_DIM, D).astype(np.float32)
